# revision 24
# baseline (speedup 1.0000x reference)
"""BiLSTM + biaffine span scorer + greedy NMS decode on 8 TRN2 NeuronCores.

Sharding: direction-split data parallelism. Cores 0-3 run the FORWARD
LSTM for sentence group [8g:8g+8]; cores 4-7 run the BACKWARD LSTM for
the same group (time-reversed inputs; identical SPMD program — the
direction lives entirely in the per-core weights/token order). Each core
runs ONE 128-step recurrence over 8 sentences (half the per-step engine
work of a two-direction kernel). Encoder halves are exchanged pairwise
(core g <-> g+4) with an AllGather; each core then runs the start/end
FFNN + 9-label biaffine + argmax for 4 sentences, selecting its rows
from the exchanged buffer via indirect DMA (keeps the program
SPMD-identical). The greedy overlap-resolving decode runs on host numpy.

Performance structure (measured on HW, ~2.2x over the two-direction
baseline):
- per-bank PSUM tiles: the tile framework tracks PSUM deps at tile
  granularity, so per-bank tiles let the activation/DVE chain overlap
  the matmul burst instead of waiting for the whole step's matmuls
- gate blocks reordered to (g, i, f, o): tanh(g)/sigmoid start as soon
  as their bank's accumulation group stops; the o-gate lands last so the
  exposed post-burst tail is just sigmoid(o) -> h -> transposes
- recurrence matmuls in float32r (single-pass, ~1.6x faster than the
  fp32 LOW+HIGH pair; tf32-ish rounding is fine for the LSTM but NOT for
  the score path, where near-tie argmax flips blow up the final error —
  FFNN/biaffine stay exact fp32)
- the x-projection add and the 16-row tail chunk of the 400-dim
  recurrence ride ONE combined matmul per gate bank (lhsT stacks
  [h-chunk3; zeros; I8], rhs stacks [Whh rows 384:400; junk; gx stage])
- c/h elementwise chain split into 256/144 halves pipelined across
  DVE/Act; h-state transposes feed two small hTr tiles so the next
  step's first matmuls start after the first cast
- PE "warmer" transposes (discarded results) keep the tensor engine's
  DVFS clock up through the elementwise tail; sustained fp32 matmuls
  otherwise throttle ~1.5x
- biaffine bias terms restructured: the x1/y1 ones-column passes (45k
  streamed PE rows) become two [*, 9]-wide matmuls + per-partition
  scalar adds on DVE
"""
import sys
sys.path.insert(0, "/opt/trn_rl_repo")
import numpy as np

VOCAB, EMB, Hh, G, L = 100000, 300, 400, 1600, 128
BL, BH, NCORES = 8, 4, 8
FF, F1, NL = 512, 513, 9
NON_ENTITY = 1

_CACHE = {}


def _build(masked):
    import concourse.bass as bass
    import concourse.mybir as mybir
    import concourse.tile as tile
    from concourse import bacc
    from concourse.masks import make_identity

    F32 = mybir.dt.float32
    F32R = mybir.dt.float32r
    I32 = mybir.dt.int32
    AF = mybir.ActivationFunctionType
    OP = mybir.AluOpType

    nc = bacc.Bacc()

    # ---------------- DRAM I/O ----------------
    emb_d = nc.dram_tensor("emb", [VOCAB, EMB], F32, kind="ExternalInput")
    idxT_d = nc.dram_tensor("idxT", [L, BL], I32, kind="ExternalInput")
    if masked:
        mask_d = nc.dram_tensor("mask", [BL, L], F32, kind="ExternalInput")
        imask_d = nc.dram_tensor("imask", [BL, L], F32, kind="ExternalInput")
    wih_d = {c: nc.dram_tensor(f"wih{c}", [rows, G], F32, kind="ExternalInput")
             for c, rows in enumerate((128, 128, 45))}
    whh_d = nc.dram_tensor("whh", [128, 3 * G], F32R, kind="ExternalInput")
    whh3_d = nc.dram_tensor("whh3", [16, G], F32R, kind="ExternalInput")
    z16_d = nc.dram_tensor("z16", [16, BL], F32R, kind="ExternalInput")
    hsel_d = nc.dram_tensor("hsel", [L, 2 * BH], I32, kind="ExternalInput")
    wsT_d = nc.dram_tensor("wsT", [100, 8, FF], F32, kind="ExternalInput")
    weT_d = nc.dram_tensor("weT", [100, 8, FF], F32, kind="ExternalInput")
    bs_d = nc.dram_tensor("bs", [1, 512], F32, kind="ExternalInput")
    be_d = nc.dram_tensor("be", [1, 512], F32, kind="ExternalInput")
    wbm_d = nc.dram_tensor("wbm", [NL, 128, 4, F1], F32, kind="ExternalInput")
    wbl_d = nc.dram_tensor("wbl", [1, NL, F1], F32, kind="ExternalInput")
    score_d = nc.dram_tensor("score_out", [BH, L, L], F32, kind="ExternalOutput")
    ans_d = nc.dram_tensor("ans_out", [BH, L, L], F32, kind="ExternalOutput")

    with tile.TileContext(nc) as tc, \
         tc.tile_pool(name="dram", bufs=1, space="DRAM") as dpool, \
         tc.tile_pool(name="ps", bufs=1, space="PSUM") as pspool, \
         tc.tile_pool(name="sb0", bufs=1) as sb0:
        gx_t = dpool.tile([L, BL, G], F32R)       # x-projections, t-major
        enc_own = dpool.tile([BL, L, Hh], F32)     # own-direction LSTM outputs
        exc_t = dpool.tile([2 * BL * L, Hh], F32)  # gathered fwd+bwd halves
        PSB = [pspool.tile([128, 512], F32, name=f"psb{i}") for i in range(8)]
        idxT = sb0.tile([L, BL], I32)
        nc.sync.dma_start(out=idxT[:], in_=idxT_d[:])
        hsel = sb0.tile([L, 2 * BH], I32)
        nc.sync.dma_start(out=hsel[:], in_=hsel_d[:])
        if masked:
            mask = sb0.tile([BL, L], F32)
            nc.sync.dma_start(out=mask[:], in_=mask_d[:])
            imask = sb0.tile([BL, L], F32)
            nc.sync.dma_start(out=imask[:], in_=imask_d[:])
        idg = sb0.tile([128, 128], F32)
        make_identity(nc, idg[:])
        ident = sb0.tile([128, 128], F32)
        nc.vector.tensor_copy(out=ident[:], in_=idg[:])
        ident8 = sb0.tile([BL, BL], F32R)
        nc.vector.tensor_copy(out=ident8[:], in_=idg[0:BL, 0:BL])
        ones1024 = sb0.tile([1, 1024], F32)
        nc.vector.memset(ones1024[:], 1.0)
        X1T = sb0.tile([128, 4, FF], F32)
        Y1T = sb0.tile([128, 4, FF], F32)
        ones = sb0.tile([1, FF], F32)
        nc.vector.tensor_copy(out=ones[:], in_=ones1024[0:1, 0:FF])

        # ================= P0/P1: gather + x-projection =================
        with tc.tile_pool(name="xp", bufs=1) as px:
            xg = []
            for b in range(BL):
                t = px.tile([L, EMB + 1], F32, name=f"xg{b}")
                nc.gpsimd.indirect_dma_start(
                    out=t[:, 0:EMB], out_offset=None, in_=emb_d[:],
                    in_offset=bass.IndirectOffsetOnAxis(ap=idxT[:, b:b + 1], axis=0))
                nc.vector.memset(t[:, EMB:EMB + 1], 1.0)
                xg.append(t)
            xT = [px.tile([128, BL * 128], F32, name="xT0"),
                  px.tile([128, BL * 128], F32, name="xT1"),
                  px.tile([45, BL * 128], F32, name="xT2")]
            for b in range(BL):
                for c, (c0, cs) in enumerate(((0, 128), (128, 128), (256, 45))):
                    po = PSB[4 + b % 4][0:cs, 0:128]
                    nc.tensor.transpose(out=po, in_=xg[b][:, c0:c0 + cs],
                                        identity=ident[:])
                    nc.vector.tensor_copy(out=xT[c][0:cs, b * 128:(b + 1) * 128],
                                          in_=po)
            wih = {}
            for c, rows in enumerate((128, 128, 45)):
                tr = px.tile([rows, G], F32, name=f"wih{c}")
                nc.sync.dma_start(out=tr[:], in_=wih_d[c][:])
                wih[c] = tr
            NCH = ((0, 512), (512, 512), (1024, 512), (1536, 64))
            for b in range(BL):
                for j, (n0, ns) in enumerate(NCH):
                    po = PSB[j % 4][:, 0:ns]
                    for c, rows in enumerate((128, 128, 45)):
                        nc.tensor.matmul(
                            out=po, lhsT=xT[c][0:rows, b * 128:(b + 1) * 128],
                            rhs=wih[c][:, n0:n0 + ns],
                            start=(c == 0), stop=(c == 2))
                    gxb = px.tile([L, 512], F32R, name="gxb", bufs=3)
                    nc.vector.tensor_copy(out=gxb[:, 0:ns], in_=po)
                    nc.gpsimd.dma_start(out=gx_t[:, b, n0:n0 + ns],
                                        in_=gxb[:, 0:ns])

        # ================= P2: LSTM (one direction, 8 sentences) =========
        with tc.tile_pool(name="lstm", bufs=1) as pw:
            whhr = pw.tile([128, 3 * G], F32R, name="whhr")
            for c in range(3):
                nc.sync.dma_start(out=whhr[:, c * G:(c + 1) * G],
                                  in_=whh_d[:, c * G:(c + 1) * G])
            # combined last-chunk operands: lhsT rows 0:16 = h chunk3 (cast
            # each step), 16:32 = zeros, 32:40 = I8 (gx add); rhs rows 0:16 =
            # Whh rows 384:400, 16:32 = junk (finite, x0), 32:40 = gx stage
            lhsC = pw.tile([40, BL], F32R, name="lhsC")
            nc.sync.dma_start(out=lhsC[16:32, :], in_=z16_d[:])
            nc.sync.dma_start(out=lhsC[0:16, :], in_=z16_d[:])
            nc.vector.tensor_copy(out=lhsC[32:40, :], in_=idg[0:BL, 0:BL])
            rhsC = [pw.tile([40, G], F32R, name=f"rhsC{i}") for i in range(2)]
            for i in range(2):
                nc.sync.dma_start(out=rhsC[i][0:16, :], in_=whh3_d[:])
                nc.sync.dma_start(out=rhsC[i][16:32, :], in_=whh3_d[:])
            hTrA0 = pw.tile([128, BL], F32R, name="hTrA0")
            hTrA1 = pw.tile([128, BL], F32R, name="hTrA1")
            hTrB = pw.tile([128, BL], F32R, name="hTrB")
            z32 = pw.tile([128, 2 * BL], F32, name="z32")
            nc.vector.memset(z32[:], 0.0)
            nc.vector.tensor_copy(out=hTrA0[:], in_=z32[:, 0:BL])
            nc.vector.tensor_copy(out=hTrA1[:], in_=z32[:, 0:BL])
            nc.vector.tensor_copy(out=hTrB[:], in_=z32[:, 0:BL])
            crow = pw.tile([BL, Hh], F32, name="crow")
            nc.vector.memset(crow[:], 0.0)
            crA = pw.tile([BL, 256], F32, name="crA")
            crB = pw.tile([BL, 144], F32, name="crB")
            nc.vector.memset(crA[:], 0.0)
            nc.vector.memset(crB[:], 0.0)
            tcA = pw.tile([BL, 256], F32, name="tcA")
            tcB = pw.tile([BL, 144], F32, name="tcB")
            if masked:
                hrow = pw.tile([BL, Hh], F32, name="hrow")
                nc.vector.memset(hrow[:], 0.0)
            S = pw.tile([BL, 3 * Hh], F32, name="S")
            T = pw.tile([BL, Hh], F32, name="T")
            m1 = pw.tile([BL, Hh], F32, name="m1")
            t2 = pw.tile([BL, Hh], F32, name="t2")
            tc_ = pw.tile([BL, Hh], F32, name="tc_")
            if masked:
                cn = pw.tile([BL, Hh], F32, name="cn")
                t3 = pw.tile([BL, Hh], F32, name="t3")
                t4 = pw.tile([BL, Hh], F32, name="t4")
                hn = pw.tile([BL, Hh], F32, name="hn")

            for t in range(L):
                bk = [PSB[(t % 2) * 4 + j] for j in range(4)]
                rc = rhsC[t % 2]
                nc.gpsimd.dma_start(out=rc[32:40, :], in_=gx_t[t, :, :])
                # gate banks: 0=g(tanh) 1=i 2=f 3=o; per bank: chunks 0-2 of
                # the recurrence, then one combined matmul carrying h-chunk3
                # AND the precomputed x-projection (via the I8 rows of lhsC)
                for j in range(4):
                    for c, hsrc_t in enumerate((hTrA0, hTrA1, hTrB)):
                        nc.tensor.matmul(
                            out=bk[j][0:BL, 0:400],
                            lhsT=hsrc_t[:, 0:BL],
                            rhs=whhr[:, c * G + j * 400:c * G + (j + 1) * 400],
                            start=(c == 0), stop=False)
                    nc.tensor.matmul(out=bk[j][0:BL, 0:400], lhsT=lhsC[:],
                                     rhs=rc[:, j * 400:(j + 1) * 400],
                                     start=False, stop=True)
                nc.scalar.activation(out=T[:], in_=bk[0][0:BL, 0:400], func=AF.Tanh)
                nc.scalar.activation(out=S[:, 0:400], in_=bk[1][0:BL, 0:400],
                                     func=AF.Sigmoid)
                nc.scalar.activation(out=S[:, 400:800], in_=bk[2][0:BL, 0:400],
                                     func=AF.Sigmoid)
                nc.scalar.activation(out=S[:, 800:928], in_=bk[3][0:BL, 0:128],
                                     func=AF.Sigmoid)
                nc.scalar.activation(out=S[:, 928:1200],
                                     in_=bk[3][0:BL, 128:400],
                                     func=AF.Sigmoid)
                nc.vector.tensor_mul(out=m1[:], in0=S[:, 0:400], in1=T[:])
                if masked:
                    m_col = mask[:, t:t + 1]
                    im_col = imask[:, t:t + 1]
                    nc.vector.tensor_mul(out=t2[:], in0=S[:, 400:800], in1=crow[:])
                    nc.vector.tensor_add(out=cn[:], in0=m1[:], in1=t2[:])
                    nc.gpsimd.tensor_scalar(out=t3[:], in0=crow[:],
                                            scalar1=im_col, scalar2=None,
                                            op0=OP.mult)
                    nc.vector.scalar_tensor_tensor(out=crow[:], in0=cn[:],
                                                   scalar=m_col, in1=t3[:],
                                                   op0=OP.mult, op1=OP.add)
                    nc.scalar.activation(out=tc_[:], in_=crow[:], func=AF.Tanh)
                    nc.vector.tensor_mul(out=hn[:], in0=S[:, 800:1200], in1=tc_[:])
                    emit = pw.tile([BL, Hh], F32, name="emit", bufs=3)
                    nc.vector.tensor_scalar(out=emit[:], in0=hn[:],
                                            scalar1=m_col, scalar2=None,
                                            op0=OP.mult)
                    nc.gpsimd.dma_start(out=enc_own[:, t, :], in_=emit[:])
                    nc.gpsimd.tensor_scalar(out=t4[:], in0=hrow[:],
                                            scalar1=im_col, scalar2=None,
                                            op0=OP.mult)
                    nc.vector.tensor_add(out=hrow[:], in0=emit[:], in1=t4[:])
                    for c in range(2):
                        nc.tensor.transpose(
                            out=bk[0][0:128, 416 + c * BL:416 + (c + 1) * BL],
                            in_=hrow[:, c * 128:(c + 1) * 128],
                            identity=ident[0:BL, 0:BL])
                    nc.tensor.transpose(out=bk[1][0:128, 416:416 + BL],
                                        in_=hrow[:, 256:384],
                                        identity=ident[0:BL, 0:BL])
                    nc.tensor.transpose(out=bk[1][0:16, 424:424 + BL],
                                        in_=hrow[:, 384:400],
                                        identity=ident[0:BL, 0:BL])
                    nc.vector.tensor_copy(out=hTrA0[:, 0:BL],
                                          in_=bk[0][0:128, 416:416 + BL])
                    nc.vector.tensor_copy(out=hTrA1[:, 0:BL],
                                          in_=bk[0][0:128, 424:424 + BL])
                    nc.vector.tensor_copy(out=hTrB[:, 0:BL],
                                          in_=bk[1][0:128, 416:416 + BL])
                    nc.vector.tensor_copy(out=lhsC[0:16, :],
                                          in_=bk[1][0:16, 424:424 + BL])
                else:
                    # PE warmers: keep the tensor-engine clock up through the
                    # elementwise tail (read T so they schedule mid-chain)
                    bkn = PSB[((t + 1) % 2) * 4 + 3]
                    for w in range(6):
                        nc.tensor.transpose(
                            out=bkn[0:128, 448:448 + BL],
                            in_=T[:, (w % 3) * 128:(w % 3) * 128 + 128],
                            identity=ident[0:BL, 0:BL])
                    emitA0 = pw.tile([BL, 128], F32, name="emitA0", bufs=3)
                    emitA1 = pw.tile([BL, 128], F32, name="emitA1", bufs=3)
                    emitB = pw.tile([BL, 144], F32, name="emitB", bufs=3)
                    nc.vector.tensor_mul(out=t2[:, 0:256], in0=S[:, 400:656],
                                         in1=crA[:])
                    nc.vector.tensor_add(out=crA[:], in0=m1[:, 0:256],
                                         in1=t2[:, 0:256])
                    nc.scalar.activation(out=tcA[:], in_=crA[:], func=AF.Tanh)
                    nc.vector.tensor_mul(out=t2[:, 256:400], in0=S[:, 656:800],
                                         in1=crB[:])
                    nc.vector.tensor_add(out=crB[:], in0=m1[:, 256:400],
                                         in1=t2[:, 256:400])
                    nc.vector.tensor_mul(out=emitA0[:], in0=S[:, 800:928],
                                         in1=tcA[:, 0:128])
                    nc.tensor.transpose(
                        out=bk[0][0:128, 416:416 + BL],
                        in_=emitA0[:], identity=ident[0:BL, 0:BL])
                    nc.vector.tensor_mul(out=emitA1[:], in0=S[:, 928:1056],
                                         in1=tcA[:, 128:256])
                    nc.vector.tensor_copy(out=hTrA0[:, 0:BL],
                                          in_=bk[0][0:128, 416:416 + BL])
                    nc.tensor.transpose(
                        out=bk[0][0:128, 424:424 + BL],
                        in_=emitA1[:], identity=ident[0:BL, 0:BL])
                    nc.scalar.activation(out=tcB[:], in_=crB[:], func=AF.Tanh)
                    nc.vector.tensor_mul(out=emitB[:], in0=S[:, 1056:1200],
                                         in1=tcB[:])
                    nc.vector.tensor_copy(out=hTrA1[:, 0:BL],
                                          in_=bk[0][0:128, 424:424 + BL])
                    nc.tensor.transpose(
                        out=bk[1][0:128, 416:416 + BL],
                        in_=emitB[:, 0:128],
                        identity=ident[0:BL, 0:BL])
                    nc.tensor.transpose(
                        out=bk[1][0:16, 424:424 + BL],
                        in_=emitB[:, 128:144],
                        identity=ident[0:BL, 0:BL])
                    nc.vector.tensor_copy(out=hTrB[:, 0:BL],
                                          in_=bk[1][0:128, 416:416 + BL])
                    nc.vector.tensor_copy(out=lhsC[0:16, :],
                                          in_=bk[1][0:16, 424:424 + BL])
                    nc.gpsimd.dma_start(out=enc_own[:, t, 0:128], in_=emitA0[:])
                    nc.gpsimd.dma_start(out=enc_own[:, t, 128:256], in_=emitA1[:])
                    nc.gpsimd.dma_start(out=enc_own[:, t, 256:400], in_=emitB[:])

        # ================= exchange: pairwise AllGather ==================
        nc.gpsimd.collective_compute(
            "AllGather", mybir.AluOpType.bypass,
            replica_groups=[[0, 4], [1, 5], [2, 6], [3, 7]],
            ins=[enc_own.opt()], outs=[exc_t.opt()])

        # ================= P3: enc gather + transpose + FFNN =============
        head_cm = tc.tile_pool(name="head", bufs=1)
        head = head_cm.__enter__()
        encT = head.tile([100, 8 * FF], F32)
        for k in range(BH):
            etile = head.tile([L, 2 * Hh], F32, name="etile", bufs=2)
            nc.gpsimd.indirect_dma_start(
                out=etile[:, 0:Hh], out_offset=None, in_=exc_t[:],
                in_offset=bass.IndirectOffsetOnAxis(
                    ap=hsel[:, 2 * k:2 * k + 1], axis=0))
            nc.gpsimd.indirect_dma_start(
                out=etile[:, Hh:2 * Hh], out_offset=None, in_=exc_t[:],
                in_offset=bass.IndirectOffsetOnAxis(
                    ap=hsel[:, 2 * k + 1:2 * k + 2], axis=0))
            for cd in range(8):
                po = PSB[4 + cd % 2][0:100, 0:128]
                nc.tensor.transpose(out=po, in_=etile[:, cd * 100:(cd + 1) * 100],
                                    identity=ident[:])
                nc.vector.tensor_copy(
                    out=encT[:, cd * FF + k * 128:cd * FF + (k + 1) * 128], in_=po)

        wsT = {}
        for nm, dram in (("s", wsT_d), ("e", weT_d)):
            tr = head.tile([100, 8, FF], F32, name=f"w{nm}r")
            nc.sync.dma_start(out=tr[:], in_=dram[:])
            wsT[nm] = tr
        bs = head.tile([1, 512], F32)
        nc.sync.dma_start(out=bs[:], in_=bs_d[:])
        be = head.tile([1, 512], F32)
        nc.sync.dma_start(out=be[:], in_=be_d[:])
        for (w_t, b_t, o_t) in ((wsT["s"], bs, X1T), (wsT["e"], be, Y1T)):
            for m in range(4):
                po = PSB[m % 4][:, 0:FF]
                for cd in range(8):
                    nc.tensor.matmul(out=po,
                                     lhsT=w_t[:, cd, m * 128:(m + 1) * 128],
                                     rhs=encT[:, cd * FF:(cd + 1) * FF],
                                     start=(cd == 0), stop=False)
                nc.tensor.matmul(out=po, lhsT=b_t[0:1, m * 128:(m + 1) * 128],
                                 rhs=ones[0:1, :], start=False, stop=True)
                nc.vector.tensor_copy(out=o_t[:, m, :], in_=po)

        head_cm.__exit__(None, None, None)

        # ================= P4: biaffine + argmax =========================
        with tc.tile_pool(name="bia", bufs=1) as pb:
            Tp = [pb.tile([128, NL, FF], F32, name=f"Tp{c}") for c in range(4)]
            Tp4 = pb.tile([1, NL, FF], F32)
            wbl = pb.tile([1, NL, F1], F32)
            nc.sync.dma_start(out=wbl[:], in_=wbl_d[:])
            for o in range(NL):
                wbo = pb.tile([128, 4, F1], F32, name="wbo", bufs=2)
                nc.sync.dma_start(out=wbo[:], in_=wbm_d[o, :, :, :])
                for mj in range(5):
                    M = 128 if mj < 4 else 1
                    po = PSB[mj % 4][0:M, 0:FF]
                    for kc in range(5):
                        if kc < 4:
                            lhsT = wbo[:, kc, mj * 128:mj * 128 + M]
                            rhs = X1T[:, kc, :]
                        else:
                            lhsT = wbl[0:1, o, mj * 128:mj * 128 + M]
                            rhs = ones[0:1, :]
                        nc.tensor.matmul(out=po, lhsT=lhsT, rhs=rhs,
                                         start=(kc == 0), stop=(kc == 4))
                    if mj < 4:
                        nc.vector.tensor_copy(out=Tp[mj][:, o, :], in_=po)
                    else:
                        nc.vector.tensor_copy(out=Tp4[:, o, :], in_=po)

            for b in range(BH):
                ps2 = (PS_B if b % 2 == 0 else PS_A)[:, 0:NL * 128]
                for n0, no in ((0, 4), (512, 4), (1024, 1)):
                    out_ap = ps2[:, n0:n0 + no * 128].rearrange(
                        "p (o x) -> p o x", o=no)
                    for kc in range(5):
                        if kc < 4:
                            lhsT = Y1T[:, kc, b * 128:(b + 1) * 128]
                            rhs = Tp[kc][:, n0 // 128:n0 // 128 + no, b * 128:(b + 1) * 128]
                        else:
                            lhsT = ones[0:1, b * 128:(b + 1) * 128]
                            rhs = Tp4[:, n0 // 128:n0 // 128 + no, b * 128:(b + 1) * 128]
                        nc.tensor.matmul(out=out_ap, lhsT=lhsT, rhs=rhs,
                                         start=(kc == 0), stop=(kc == 4))
                m_t = pb.tile([128, 128], F32, name="m_t", bufs=2)
                nc.vector.tensor_reduce(
                    out=m_t[:],
                    in_=ps2.rearrange("p (o x) -> p x o", o=NL),
                    axis=mybir.AxisListType.X, op=OP.max)
                vm = pb.tile([128, 128], F32, name="vm", bufs=2)
                eq = pb.tile([128, 128], F32, name="eq", bufs=2)
                to_ = pb.tile([128, 128], F32, name="to_", bufs=2)
                for o in range(NL):
                    nc.vector.tensor_tensor(out=eq[:], in0=ps2[:, o * 128:(o + 1) * 128],
                                            in1=m_t[:], op=OP.is_equal)
                    if o == 0:
                        nc.vector.tensor_scalar(out=vm[:], in0=eq[:], scalar1=-1000.0,
                                                scalar2=float(o), op0=OP.mult,
                                                op1=OP.add)
                    else:
                        nc.vector.tensor_scalar(out=to_[:], in0=eq[:], scalar1=-1000.0,
                                                scalar2=float(o), op0=OP.mult,
                                                op1=OP.add)
                        nc.vector.tensor_tensor(out=vm[:], in0=vm[:], in1=to_[:],
                                                op=OP.min)
                ans_t = pb.tile([128, 128], F32, name="ans_t", bufs=2)
                nc.vector.tensor_scalar(out=ans_t[:], in0=vm[:], scalar1=1000.0,
                                        scalar2=None, op0=OP.add)
                nc.gpsimd.dma_start(out=score_d[b, :, :], in_=m_t[:])
                nc.gpsimd.dma_start(out=ans_d[b, :, :], in_=ans_t[:])

    nc.finalize()
    return nc


# Gate block permutation: torch order (i, f, g, o) -> kernel order (g, i, f, o)
_PERM = (2, 0, 1, 3)


def _permute_gates(M):
    return np.concatenate([M[400 * p:400 * (p + 1)] for p in _PERM], axis=0)


def _host_prep(inputs):
    """Per-core input maps from the full inputs."""
    f32 = np.float32
    word_idxs = np.asarray(inputs["word_idxs"])
    emb = np.ascontiguousarray(np.asarray(inputs["word_emb"], dtype=f32))

    def wpack(Wih, Whh, bih, bhh):
        Wihp = _permute_gates(np.asarray(Wih, f32))
        Whhp = _permute_gates(np.asarray(Whh, f32))
        bias = _permute_gates((np.asarray(bih, f32) + np.asarray(bhh, f32))[:, None])[:, 0]
        wih_aug = np.concatenate([Wihp.T, bias[None, :]], axis=0)  # [301, 1600]
        chunks = [np.ascontiguousarray(wih_aug[0:128]),
                  np.ascontiguousarray(wih_aug[128:256]),
                  np.ascontiguousarray(wih_aug[256:301])]
        whhT = Whhp.T  # [400, 1600]
        whh_p = np.ascontiguousarray(
            np.concatenate([whhT[c * 128:(c + 1) * 128] for c in range(3)], axis=1))
        whh3 = np.ascontiguousarray(whhT[384:400])
        return chunks, whh_p, whh3

    wf, whf, wh3f = wpack(inputs["Wih_f"], inputs["Whh_f"], inputs["bih_f"], inputs["bhh_f"])
    wb, whb, wh3b = wpack(inputs["Wih_b"], inputs["Whh_b"], inputs["bih_b"], inputs["bhh_b"])

    def ffpack(W):  # [512, 800] -> [100, 8, 512]
        WT = np.asarray(W, f32).T  # [800, 512]
        return np.ascontiguousarray(
            np.stack([WT[c * 100:(c + 1) * 100] for c in range(8)], axis=1))

    wsT = ffpack(inputs["W_start"])
    weT = ffpack(inputs["W_end"])
    bs = np.ascontiguousarray(np.asarray(inputs["b_start"], f32)[None, :])
    be = np.ascontiguousarray(np.asarray(inputs["b_end"], f32)[None, :])
    Wb = np.asarray(inputs["W_biaffine"], f32)  # [9, 513, 513]
    wbm = np.ascontiguousarray(
        np.stack([np.stack([Wb[o, kc * 128:(kc + 1) * 128, :] for kc in range(4)],
                           axis=0) for o in range(NL)], axis=0))  # [9,4,128,513]
    wbm = np.ascontiguousarray(wbm.transpose(0, 2, 1, 3))  # [9,128,4,513]
    wbl = np.ascontiguousarray(Wb[:, 512, :][None, :, :])  # [1,9,513]

    # head-phase gather indices into exc_t [2*BL*L, Hh]
    tarr = np.arange(L)

    def hsel_for(first_half):
        h = np.empty((L, 2 * BH), np.int32)
        for k in range(BH):
            sk = k if first_half else BH + k
            h[:, 2 * k] = sk * L + tarr                    # fwd part, t order
            h[:, 2 * k + 1] = (BL + sk) * L + (L - 1 - tarr)  # bwd part, reversed
        return np.ascontiguousarray(h)

    hsel_f = hsel_for(True)
    hsel_b = hsel_for(False)

    shared = {"emb": emb, "wsT": wsT, "weT": weT, "bs": bs, "be": be,
              "wbm": wbm, "wbl": wbl}
    masked = bool((word_idxs <= 0).any())
    in_maps = []
    for core in range(NCORES):
        g = core % 4
        fwd = core < 4
        sl = word_idxs[g * BL:(g + 1) * BL]  # [8, 128]
        slp = sl if fwd else sl[:, ::-1]
        idxT = np.ascontiguousarray(slp.T.astype(np.int32))
        d = dict(shared)
        d["idxT"] = idxT
        m = (slp > 0).astype(f32)
        d["mask"] = np.ascontiguousarray(m)
        d["imask"] = np.ascontiguousarray(1.0 - m)
        chunks, whh_p, whh3 = (wf, whf, wh3f) if fwd else (wb, whb, wh3b)
        d["wih0"], d["wih1"], d["wih2"] = chunks
        d["whh"] = whh_p
        d["whh3"] = whh3
        d["z16"] = np.zeros((16, BL), np.float32)
        d["hsel"] = hsel_f if fwd else hsel_b
        in_maps.append(d)
    return in_maps, masked


def _decode_one(score, ans, labels):
    """Exact skip-based equivalent of the reference greedy scan."""
    Ls = L
    valid = (ans != NON_ENTITY) & (labels > 0)
    flat = np.where(valid, score, -np.inf).ravel()
    alive = valid.ravel().copy()
    res = np.full((Ls, Ls), NON_ENTITY, np.int32)
    start = np.zeros(Ls, bool)
    inside = np.zeros(Ls, bool)
    ii = np.arange(Ls)[:, None]
    jj = np.arange(Ls)[None, :]
    while alive.any():
        cs = np.cumsum(start)
        csm1 = np.concatenate(([0], cs[:-1]))
        cnt = cs[None, :] - csm1[:, None]
        conflict = ((ii <= jj) & (cnt > 0)) | inside[:, None]
        cand = alive & ~conflict.ravel()
        if not cand.any():
            break
        f = np.where(cand, flat, -np.inf)
        k = int(np.argmax(f))
        if f[k] == -np.inf:
            break
        i, j = divmod(k, Ls)
        start[i] = True
        if i <= j:
            inside[i:j + 1] = True
        res[i, j] = ans[i, j]
        alive[k] = False
    return res


def kernel(**inputs):
    from concourse.bass_utils import run_bass_kernel_spmd

    in_maps, masked = _host_prep(inputs)
    key = ("nc", masked)
    if key not in _CACHE:
        _CACHE[key] = _build(masked)
        _CACHE["nc"] = _CACHE[key]
    nc = _CACHE[key]

    if not masked:
        for d in in_maps:
            d.pop("mask")
            d.pop("imask")

    res = run_bass_kernel_spmd(nc, in_maps, core_ids=list(range(NCORES)))

    labels = np.asarray(inputs["labels"])
    out = np.empty((NCORES * BH, L, L), np.int32)
    for core in range(NCORES):
        g = core % 4
        base = g * BL + (0 if core < 4 else BH)
        r = res.results[core]
        for b in range(BH):
            s = r["score_out"][b].T          # [y,x] -> [x,y]
            a = np.rint(r["ans_out"][b].T).astype(np.int32)
            out[base + b] = _decode_one(s, a, labels[base + b])
    return out
